# revision 1
# baseline (speedup 1.0000x reference)
"""DiffAttention TRN2 kernel: 8-core SPMD (batch x head-group sharding).

Sharding: core c -> batch b = c//4, head-group g = c%4 (4 raw heads = 2
effective pairs per core). Each core computes q/k/v projections for its
head slice, diff-attention, headwise RMSNorm, and a partial output
projection (its 256 input-feature slice of Wo). Host sums the 4 partials
per batch and adds bo.

Device layout: feature-major ("transposed") activations so the PE
contraction always runs along partitions:
  xT [1024, T] via PE transposes; qT/kT [256, T] f32r; v [T, 256] bf16;
  scores k-major sT [128k, 512q] (lhsT=kT head slice, row-packed 2 heads);
  exp -> bf16 e-tiles; softmax denominators via col-packed M=1 bf16
  ones-matmuls (dst partitions 0/32); attn-out oT = v.T @ e accumulated
  over k-blocks in PSUM; RMSNorm via ones-matmul partition sum-of-squares
  + exp(-0.5*ln(ms)); out projection from feature-major oT_n.
"""

import sys

sys.path.insert(0, "/opt/trn_rl_repo")

import numpy as np

import concourse.bass as bass  # noqa: F401
import concourse.mybir as mybir
import concourse.tile as tile
from concourse import bacc
from concourse.bass_utils import run_bass_kernel_spmd
from concourse.masks import make_identity

# Steer every activation to the one table set containing both Exp and Ln
# (no ACT table thrash). Set order must stay identical to act_info.json --
# the emitted act_func_set_id is positional -- so instead of reordering we
# hide the functions we use from every other set so the chooser can't pick
# them.
_orig_gat = bacc.get_activation_tables
_PREF_SET = "natural_log_exp_and_others"
def _gat_pref(arch):
    tabs = _orig_gat(arch)
    if _PREF_SET not in tabs:
        return tabs
    used = {mybir.ActivationFunctionType.Exp, mybir.ActivationFunctionType.Ln,
            mybir.ActivationFunctionType.Identity, mybir.ActivationFunctionType.Copy,
            mybir.ActivationFunctionType.Square}
    out = {}
    for name, fns in tabs.items():
        if name == _PREF_SET:
            out[name] = fns
        else:
            out[name] = {f for f in fns if f not in used}
    return out
bacc.get_activation_tables = _gat_pref

F32 = mybir.dt.float32
F32R = mybir.dt.float32r
BF16 = mybir.dt.bfloat16
AL = mybir.AluOpType
AF = mybir.ActivationFunctionType

B, N, DIM = 2, 2048, 1024
NUM_HEADS = 16
EFF = NUM_HEADS // 2
HD = DIM // NUM_HEADS          # 64
HPC = 4                        # raw heads per core
PPC = 2                        # head pairs per core
DLOC = HPC * HD                # 256 local feature dims
LAMBDA_INIT = 0.8
EPS = 1e-5
SCALE = HD ** -0.5

NT = N // 128                  # 16 token tiles of 128
NC4 = N // 512                 # 4 chunks of 512 tokens
KB = N // 128                  # 16 k-blocks

_CACHE = {}


def _build_nc():
    nc = bacc.Bacc()

    x_ = nc.declare_dram_parameter("x", [N, DIM], F32, isOutput=False)
    wqT = nc.declare_dram_parameter("wqT", [DIM, DLOC], F32, isOutput=False)
    wkT = nc.declare_dram_parameter("wkT", [DIM, DLOC], F32, isOutput=False)
    wvT = nc.declare_dram_parameter("wvT", [DIM, DLOC], F32, isOutput=False)
    woT = nc.declare_dram_parameter("woT", [DLOC, DIM], F32, isOutput=False)
    bq_ = nc.declare_dram_parameter("bq", [DLOC], F32, isOutput=False)
    bk_ = nc.declare_dram_parameter("bk", [DLOC], F32, isOutput=False)
    bv_ = nc.declare_dram_parameter("bv", [DLOC], F32, isOutput=False)
    nw_ = nc.declare_dram_parameter("nw", [2 * HD], F32, isOutput=False)
    lq1_ = nc.declare_dram_parameter("lq1", [HD], F32, isOutput=False)
    lk1_ = nc.declare_dram_parameter("lk1", [HD], F32, isOutput=False)
    lq2_ = nc.declare_dram_parameter("lq2", [HD], F32, isOutput=False)
    lk2_ = nc.declare_dram_parameter("lk2", [HD], F32, isOutput=False)
    out_ = nc.declare_dram_parameter("out", [N, DIM], F32, isOutput=True)

    with tile.TileContext(nc) as tc:
        with tc.tile_pool(name="persist", bufs=1) as pp:
            # ---- constants / weights ----
            ident = pp.tile([128, 128], F32, tag="ident")
            make_identity(nc, ident[:])
            ones_bf = pp.tile([128, 1], BF16, tag="ones")
            nc.vector.memset(ones_bf[:], 1.0)

            # ================= PHASE A pools (opened early so the x prefetch
            # can precede the 4MB of weight DMAs in queue order) ============
            apools = (
                tc.tile_pool(name="xin", bufs=5),
                tc.tile_pool(name="xt", bufs=10),
                tc.tile_pool(name="tp_ps", bufs=3, space="PSUM"),
                tc.tile_pool(name="pj_ps", bufs=3, space="PSUM"),
            )
            xin_pool, xt_pool, tp_ps, pj_ps = (p.__enter__() for p in apools)
            xin_pool, xt_pool, tp_ps, pj_ps = [p for p in (xin_pool, xt_pool, tp_ps, pj_ps)]

            x0_rows = []
            for tt in range(4):
                xr = xin_pool.tile([128, DIM], F32, tag="xin", name="xr0")
                nc.sync.dma_start(xr[:], x_[tt * 128:(tt + 1) * 128, :])
                x0_rows.append(xr)

            wq_t = []
            wk_t = []
            wv_t = []
            for i in range(8):
                t = pp.tile([128, DLOC], F32R, tag=f"wq{i}")
                nc.scalar.dma_start(t[:], wqT[i * 128:(i + 1) * 128, :].bitcast(F32R))
                wq_t.append(t)
                t = pp.tile([128, DLOC], F32R, tag=f"wk{i}")
                nc.scalar.dma_start(t[:], wkT[i * 128:(i + 1) * 128, :].bitcast(F32R))
                wk_t.append(t)
                t = pp.tile([128, DLOC], F32R, tag=f"wv{i}")
                nc.scalar.dma_start(t[:], wvT[i * 128:(i + 1) * 128, :].bitcast(F32R))
                wv_t.append(t)
            wo_t = []
            for p in range(PPC):
                t = pp.tile([128, DIM], F32R, tag=f"wo{p}")
                nc.scalar.dma_start(t[:], woT[p * 128:(p + 1) * 128, :].bitcast(F32R))
                wo_t.append(t)

            bq_t = []
            bk_t = []
            for fc in range(2):
                t = pp.tile([128, 1], F32, tag=f"bq{fc}")
                nc.sync.dma_start(t[:], bq_[fc * 128:(fc + 1) * 128].rearrange("(p one) -> p one", one=1))
                bq_t.append(t)
                t = pp.tile([128, 1], F32, tag=f"bk{fc}")
                nc.sync.dma_start(t[:], bk_[fc * 128:(fc + 1) * 128].rearrange("(p one) -> p one", one=1))
                bk_t.append(t)
            bv_row = pp.tile([1, DLOC], F32, tag="bvrow")
            nc.sync.dma_start(bv_row[:], bv_[:].rearrange("(one f) -> one f", one=1))
            bv_bc = pp.tile([128, DLOC], F32, tag="bvbc")
            nc.gpsimd.partition_broadcast(bv_bc[:], bv_row[:])

            nw_t = pp.tile([128, 1], F32, tag="nw")
            nc.sync.dma_start(nw_t[:], nw_[:].rearrange("(p one) -> p one", one=1))
            nw02 = pp.tile([128, 1], F32, tag="nw02")
            nc.scalar.mul(nw02[:], nw_t[:], 1.0 - LAMBDA_INIT)

            # ---- lambda scalar on device ----
            lrow = pp.tile([1, 4 * HD], F32, tag="lrow")
            for j, lp in enumerate([lq1_, lk1_, lq2_, lk2_]):
                nc.sync.dma_start(lrow[:, j * HD:(j + 1) * HD], lp[:].rearrange("(one f) -> one f", one=1))
            lprod = pp.tile([1, 2 * HD], F32, tag="lprod")
            nc.vector.tensor_mul(lprod[:, 0:HD], lrow[:, 0:HD], lrow[:, HD:2 * HD])
            nc.vector.tensor_mul(lprod[:, HD:2 * HD], lrow[:, 2 * HD:3 * HD], lrow[:, 3 * HD:4 * HD])
            lsum = pp.tile([1, 2], F32, tag="lsum")
            nc.vector.tensor_reduce(lsum[:, 0:1], lprod[:, 0:HD], mybir.AxisListType.X, AL.add)
            nc.vector.tensor_reduce(lsum[:, 1:2], lprod[:, HD:2 * HD], mybir.AxisListType.X, AL.add)
            lexp = pp.tile([1, 2], F32, tag="lexp")
            nc.scalar.activation(lexp[:], lsum[:], AF.Exp)
            lam_t = pp.tile([1, 1], F32, tag="lam")
            nc.vector.tensor_sub(lam_t[:], lexp[:, 0:1], lexp[:, 1:2])
            nc.vector.tensor_scalar_add(lam_t[:], lam_t[:], LAMBDA_INIT)

            # ---- persistent activations ----
            qT_sb = [pp.tile([128, N], F32R, tag=f"qT{fc}", name=f"qT{fc}") for fc in range(2)]
            kT_sb = [pp.tile([128, N], F32R, tag=f"kT{fc}", name=f"kT{fc}") for fc in range(2)]
            v_sb = [pp.tile([128, DLOC], BF16, tag=f"v{tt}", name=f"v{tt}") for tt in range(NT)]
            oTn = [pp.tile([128, N], F32R, tag=f"oTn{p}", name=f"oTn{p}") for p in range(PPC)]

            # ================= PHASE A: xT + projections =================
            if True:
                def emit_vproj(tc4, xts):
                    for tt in range(4):
                        ps = pj_ps.tile([128, DLOC], F32, tag="pj")
                        for i in range(8):
                            nc.tensor.matmul(
                                ps[:],
                                xts[i][:, tt * 128:(tt + 1) * 128],
                                wv_t[i][:],
                                start=(i == 0),
                                stop=(i == 7),
                            )
                        nc.vector.scalar_tensor_tensor(
                            v_sb[tc4 * 4 + tt][:], ps[:], 1.0, bv_bc[:], AL.mult, AL.add)

                for tc4 in range(NC4):
                    tsl = slice(tc4 * 512, (tc4 + 1) * 512)
                    # load x chunk as 4 token tiles, transpose to xT [128i, 512t]
                    if tc4 == 0:
                        xrows = x0_rows
                    else:
                        xrows = []
                        for tt in range(4):
                            xr = xin_pool.tile([128, DIM], F32, tag="xin")
                            nc.sync.dma_start(xr[:], x_[tc4 * 512 + tt * 128: tc4 * 512 + (tt + 1) * 128, :])
                            xrows.append(xr)
                    xts = []
                    for i in range(8):
                        xtp = tp_ps.tile([128, 512], F32, tag="tp")
                        for tt in range(4):
                            nc.tensor.transpose(
                                xtp[:, tt * 128:(tt + 1) * 128],
                                xrows[tt][:, i * 128:(i + 1) * 128],
                                ident[:],
                            )
                        xt = xt_pool.tile([128, 512], F32R, tag="xt")
                        nc.vector.tensor_copy(xt[:], xtp[:])
                        xts.append(xt)
                    # q/k projections (feature-major)
                    for wlist, dest, bias, scl in (
                        (wq_t, qT_sb, bq_t, SCALE),
                        (wk_t, kT_sb, bk_t, 1.0),
                    ):
                        for fc in range(2):
                            ps = pj_ps.tile([128, 512], F32, tag="pj")
                            for i in range(8):
                                nc.tensor.matmul(
                                    ps[:],
                                    wlist[i][:, fc * 128:(fc + 1) * 128],
                                    xts[i][:],
                                    start=(i == 0),
                                    stop=(i == 7),
                                )
                            nc.vector.tensor_scalar(dest[fc][:, tsl], ps[:], scl, bias[fc][:],
                                                    AL.mult, AL.add)
                    emit_vproj(tc4, xts)

            for p_ in reversed(apools):
                p_.__exit__(None, None, None)

            # ========== PHASE B: attention + norm + output projection ==========
            with (
                tc.tile_pool(name="s_ps", bufs=3, space="PSUM") as s_ps,
                tc.tile_pool(name="a_ps", bufs=3, space="PSUM") as a_ps,
                tc.tile_pool(name="d_ps", bufs=2, space="PSUM") as d_ps,
                tc.tile_pool(name="e_sb", bufs=6) as e_pool,
                tc.tile_pool(name="cmb", bufs=4) as cmb_pool,
                tc.tile_pool(name="row", bufs=6) as row_pool,
                tc.tile_pool(name="bc", bufs=4) as bc_pool,
                tc.tile_pool(name="o_sb", bufs=4) as o_sb,
            ):
                def emit_combine1(p, qc, A0, A1, dacc):
                    """Part 1: softmax-normalize + diff-combine (reads/frees
                    the A psum banks)."""
                    inv0 = row_pool.tile([1, 512], F32, tag="row", name="inv0")
                    nc.vector.reciprocal(inv0[:], dacc[0:1, :])
                    inv1 = row_pool.tile([1, 512], F32, tag="row", name="inv1")
                    nc.vector.reciprocal(inv1[:], dacc[32:33, :])
                    inv1l = row_pool.tile([1, 512], F32, tag="row", name="inv1l")
                    nc.vector.tensor_scalar(inv1l[:], inv1[:], lam_t[0:1, 0:1], None, AL.mult, AL.bypass)
                    bc0 = bc_pool.tile([128, 512], F32, tag="bc", name="bc0")
                    nc.gpsimd.partition_broadcast(bc0[:], inv0[:])
                    bc1 = bc_pool.tile([128, 512], F32, tag="bc", name="bc1")
                    nc.gpsimd.partition_broadcast(bc1[:], inv1l[:])
                    m0 = cmb_pool.tile([128, 512], F32, tag="m", name="m0")
                    nc.vector.tensor_mul(m0[:], A0[:], bc0[:])
                    m1 = cmb_pool.tile([128, 512], F32, tag="m", name="m1")
                    nc.vector.tensor_mul(m1[:], A1[:], bc1[:])
                    ou = cmb_pool.tile([128, 512], F32, tag="ou", name="ou", bufs=3)
                    nc.vector.tensor_sub(ou[:], m0[:], m1[:])
                    return ou

                def emit_combine2(p, qc, ou, dacc):
                    """Part 2: headwise RMSNorm; writes oTn[p][:, qc]."""
                    qsl = slice(qc * 512, (qc + 1) * 512)
                    sq = e_pool.tile([128, 512], BF16, tag="sq", name="sq", bufs=2)
                    nc.vector.tensor_mul(sq[:], ou[:], ou[:])
                    nc.tensor.matmul(dacc[64:65, :], ones_bf[:], sq[:], start=True, stop=True,
                                     tile_position=(0, 64), skip_group_check=True)
                    msn = row_pool.tile([1, 512], F32, tag="row", name="msn")
                    nc.vector.tensor_scalar(msn[:], dacc[64:65, :], 1.0 / (2 * HD), EPS, AL.mult, AL.add)
                    lnm = row_pool.tile([1, 512], F32, tag="row", name="lnm")
                    nc.scalar.activation(lnm[:], msn[:], AF.Ln)
                    rstd = row_pool.tile([1, 512], F32, tag="row", name="rstd")
                    nc.scalar.activation(rstd[:], lnm[:], AF.Exp, scale=-0.5)
                    bcr = bc_pool.tile([128, 512], F32, tag="bc", name="bcr")
                    nc.gpsimd.partition_broadcast(bcr[:], rstd[:])
                    tmp = cmb_pool.tile([128, 512], F32, tag="m", name="ntmp")
                    nc.vector.tensor_mul(tmp[:], ou[:], bcr[:])
                    nc.scalar.activation(oTn[p][:, qsl], tmp[:], AF.Identity, scale=nw02[:])

                def emit_outproj(qc):
                    for tt4 in range(4):
                        tt = qc * 4 + tt4
                        tsl = slice(tt * 128, (tt + 1) * 128)
                        for oc in range(2):
                            osl = slice(oc * 512, (oc + 1) * 512)
                            ps = a_ps.tile([128, 512], F32, tag="A", name="ops")
                            for p in range(PPC):
                                nc.tensor.matmul(ps[:], oTn[p][:, tsl], wo_t[p][:, osl],
                                                 start=(p == 0), stop=(p == PPC - 1))
                            ot = o_sb.tile([128, 512], F32, tag="ot", name="ot")
                            nc.vector.tensor_copy(ot[:], ps[:])
                            nc.sync.dma_start(out_[tsl, osl], ot[:])

                pending = None
                pending2 = None
                for qc in range(NC4):
                    qsl = slice(qc * 512, (qc + 1) * 512)
                    for p in range(PPC):
                        A0 = a_ps.tile([128, 512], F32, tag="A", name="A0")
                        A1 = a_ps.tile([128, 512], F32, tag="A", name="A1")
                        dacc = d_ps.tile([128, 512], F32, tag="d", name="dacc")
                        for kb in range(KB):
                            ksl = slice(kb * 128, (kb + 1) * 128)
                            s0 = s_ps.tile([128, 512], F32, tag="s", name="s0")
                            s1 = s_ps.tile([128, 512], F32, tag="s", name="s1")
                            nc.tensor.matmul(s0[:], kT_sb[p][0:64, ksl], qT_sb[p][0:64, qsl],
                                             start=True, stop=True)
                            nc.tensor.matmul(s1[:], kT_sb[p][64:128, ksl], qT_sb[p][64:128, qsl],
                                             start=True, stop=True)
                            e0t = e_pool.tile([128, 512], BF16, tag="e", name="e0")
                            nc.scalar.activation(e0t[:], s0[:], AF.Exp)
                            e1t = e_pool.tile([128, 512], BF16, tag="e", name="e1")
                            nc.scalar.activation(e1t[:], s1[:], AF.Exp)
                            e0 = e0t[:]
                            e1 = e1t[:]
                            st, sp = (kb == 0), (kb == KB - 1)
                            vt = v_sb[kb][:, p * 128:(p + 1) * 128]
                            nc.tensor.matmul(A0[:], vt, e0, start=st, stop=sp)
                            nc.tensor.matmul(A1[:], vt, e1, start=st, stop=sp)
                            nc.tensor.matmul(dacc[0:1, :], ones_bf[:], e0, start=st, stop=sp,
                                             tile_position=(0, 0), skip_group_check=True)
                            nc.tensor.matmul(dacc[32:33, :], ones_bf[:], e1, start=st, stop=sp,
                                             tile_position=(0, 32), skip_group_check=True)
                            if kb == 1 and pending is not None:
                                pp_, pqc, pA0, pA1, pdacc = pending
                                pou = emit_combine1(pp_, pqc, pA0, pA1, pdacc)
                                pending = None
                                pending2 = (pp_, pqc, pou, pdacc)
                            if kb == 5 and pending2 is not None:
                                emit_combine2(*pending2)
                                pending2 = None
                            if kb == 9 and p == 0 and qc > 0:
                                emit_outproj(qc - 1)
                        pending = (p, qc, A0, A1, dacc)
                pp_, pqc, pA0, pA1, pdacc = pending
                pou = emit_combine1(pp_, pqc, pA0, pA1, pdacc)
                emit_combine2(pp_, pqc, pou, pdacc)
                emit_outproj(pqc)


    nc.finalize()
    return nc


def kernel(x, Wq, bq, Wk, bk, Wv, bv, Wo, bo, norm_w, lq1, lk1, lq2, lk2):
    x = np.asarray(x, dtype=np.float32)
    Wq = np.asarray(Wq, dtype=np.float32)
    Wk = np.asarray(Wk, dtype=np.float32)
    Wv = np.asarray(Wv, dtype=np.float32)
    Wo = np.asarray(Wo, dtype=np.float32)

    if "nc" not in _CACHE:
        _CACHE["nc"] = _build_nc()
    nc = _CACHE["nc"]

    in_maps = []
    for c in range(8):
        b, g = c // 4, c % 4
        sl = slice(g * DLOC, (g + 1) * DLOC)
        in_maps.append({
            "x": np.ascontiguousarray(x[b]),
            "wqT": np.ascontiguousarray(Wq[sl, :].T),
            "wkT": np.ascontiguousarray(Wk[sl, :].T),
            "wvT": np.ascontiguousarray(Wv[sl, :].T),
            "woT": np.ascontiguousarray(Wo[:, sl].T),
            "bq": np.ascontiguousarray(np.asarray(bq, np.float32)[sl]),
            "bk": np.ascontiguousarray(np.asarray(bk, np.float32)[sl]),
            "bv": np.ascontiguousarray(np.asarray(bv, np.float32)[sl]),
            "nw": np.ascontiguousarray(np.asarray(norm_w, np.float32)),
            "lq1": np.ascontiguousarray(np.asarray(lq1, np.float32)),
            "lk1": np.ascontiguousarray(np.asarray(lk1, np.float32)),
            "lq2": np.ascontiguousarray(np.asarray(lq2, np.float32)),
            "lk2": np.ascontiguousarray(np.asarray(lk2, np.float32)),
        })

    res = None
    for attempt in range(3):
        try:
            res = run_bass_kernel_spmd(nc, in_maps, list(range(8))).results
            break
        except Exception:
            # transient device wedge (NRT_EXEC_UNIT_UNRECOVERABLE): the PJRT
            # client can hold a dead handle, so tear the backend down and
            # reconnect before retrying
            if attempt == 2:
                raise
            import time as _time

            _time.sleep(5 + 10 * attempt)
            try:
                import jax

                jax.clear_caches()
                jax.extend.backend.clear_backends()
            except Exception:
                pass
    bo_f = np.asarray(bo, np.float32)
    out = np.empty((B, N, DIM), np.float32)
    for b in range(B):
        acc = res[4 * b]["out"].astype(np.float32)
        for g in range(1, 4):
            acc = acc + res[4 * b + g]["out"]
        out[b] = acc + bo_f[None, :]
    return out

